# revision 32
# baseline (speedup 1.0000x reference)
"""KNRM ranking kernel for 8 Trainium2 NeuronCores.

Data-parallel over batch (1024 -> 8 x 128). The normalized embedding table is
shipped as fp16 rounded to 6 mantissa bits and split into byte planes (the
axon transport zstd-compresses jit args, and byte-planed 12-bit-significant
fp16 compresses ~1.6x), row-sharded (12500 rows/core) over the slow host link,
AllGathered on device over NeuronLink, then byte-merged into an fp16 table.
The exact-match kernel bin (mu=1, sigma=0.001) is computed from integer id
equality on device (DVE is_equal), so table quantization cannot break it.
Per core:
  - gather fp16 embeddings for query/doc token ids (indirect DMA)
  - PE-transpose gathered tiles so the embed dim is on partitions
  - cosine sim = fp16 matmul of normalized embeddings (simT layout [d,(b,q)])
  - soft histogram: exp(-(s-mu_k)^2/(2 sigma_k^2)) for the 10 sigma=0.1 bins,
    factorized as U(s)*V_k(s) with U = exp(-50 s^2), V_k = exp(100 mu_k s -
    50 mu_k^2); the exact bin from id equality.
  - sum over doc dim via PE ones-selector matmuls into PSUM, log1p via ACT
    Log(bias=1), MLP dot via PE, query-sum via DVE reduce, sigmoid via exp+recip.

All imports, the Bass build, the NEFF compile, and jax/axon/comm init happen at
module import (with a dummy-input warmup), so kernel() itself only pays host
prep + input transfer + execution.
"""

import os
from contextlib import ExitStack

import numpy as np

LAST_RESULT = None

B, QLEN, DLEN, EMBED, VOCAB, NK = 1024, 32, 256, 128, 100000, 11
NCORES = 8
VSHARD = VOCAB // NCORES  # 12500 rows per core
BLOC = B // NCORES  # 128
NGRP = BLOC // 4    # 32 groups of 4 batch items
NSC = 4             # super-chunks per pass (8 groups each)
GPS = NGRP // NSC   # 8 groups per super-chunk
SCCOLS = GPS * 128  # 1024 unique (b,q) cols per super-chunk
XCOLS = 2 * SCCOLS  # 2048 incl. both doc halves

DE_TILES = BLOC * 2           # 256 de gather tiles per pass
QE_TILES = NGRP               # 32 qe gather tiles per pass
TILES_PER_PASS = DE_TILES + QE_TILES  # 288
IDS_COLS = 2 * TILES_PER_PASS
QROW_COLS = 2 * NGRP * 128    # 8192: q-ids in (b,q) column order, both passes

_MUS = [-0.9, -0.7, -0.5, -0.3, -0.1, 0.1, 0.3, 0.5, 0.7, 0.9]  # sigma=0.1 bins


def _build_nc():
    import concourse.bass as bass
    import concourse.mybir as mybir
    import concourse.tile as tile
    from concourse import bacc
    from concourse.masks import make_identity

    f32 = mybir.dt.float32
    f16 = mybir.dt.float16
    u8 = mybir.dt.uint8
    i32 = mybir.dt.int32
    EXP = mybir.ActivationFunctionType.Exp
    LOG = mybir.ActivationFunctionType.Ln
    ADD = mybir.AluOpType.add
    ISEQ = mybir.AluOpType.is_equal
    AXX = mybir.AxisListType.X

    nc = bacc.Bacc(None, target_bir_lowering=False, num_devices=NCORES)
    with tile.TileContext(nc) as tc, ExitStack() as ctx:
        dram = ctx.enter_context(tc.tile_pool(name="dram", bufs=1, space="DRAM"))
        # per core: 12-bit packed m6 fp16 shard: 0.8MB paired lo-nibbles,
        # then 1.6MB hi bytes (sign+exp+2 mantissa bits)
        EMB_IN_ROWS = 3 * VSHARD // 2  # 18750 rows of 128 = 2.4MB
        emb_in = dram.tile([EMB_IN_ROWS, EMBED], u8, kind="ExternalInput")
        # ids as 3 byte planes (ids < 100000 < 2^17; plane 3 is always 0)
        ids = dram.tile([128, 3 * IDS_COLS], u8, kind="ExternalInput")
        qrow = dram.tile([1, QROW_COLS], i32, kind="ExternalInput")
        wvec = dram.tile([NK, 1], f32, kind="ExternalInput")
        out = dram.tile([1, BLOC], f32, kind="ExternalOutput")

        # collective bounce buffers (collectives can't touch I/O tensors)
        shard_b = dram.tile([EMB_IN_ROWS, EMBED], u8)
        gat = dram.tile([NCORES * EMB_IN_ROWS, EMBED], u8, addr_space="Shared")
        table = dram.tile([VOCAB, EMBED], f16)

        const = ctx.enter_context(tc.tile_pool(name="const", bufs=1))
        gde = ctx.enter_context(tc.tile_pool(name="gde", bufs=6))
        gqe = ctx.enter_context(tc.tile_pool(name="gqe", bufs=3))
        tps = ctx.enter_context(tc.tile_pool(name="tps", bufs=2, space="PSUM"))
        det = ctx.enter_context(tc.tile_pool(name="det", bufs=6))
        qet = ctx.enter_context(tc.tile_pool(name="qet", bufs=3))
        sps = ctx.enter_context(tc.tile_pool(name="sps", bufs=2, space="PSUM"))
        xp = ctx.enter_context(tc.tile_pool(name="xp", bufs=2))
        up = ctx.enter_context(tc.tile_pool(name="up", bufs=2))
        vp = ctx.enter_context(tc.tile_pool(name="vp", bufs=3))
        pp = ctx.enter_context(tc.tile_pool(name="pp", bufs=3))
        pooled = ctx.enter_context(tc.tile_pool(name="pooled", bufs=1, space="PSUM"))
        lgt = ctx.enter_context(tc.tile_pool(name="lgt", bufs=2, space="PSUM"))
        lp = ctx.enter_context(tc.tile_pool(name="lp", bufs=2))
        fp = ctx.enter_context(tc.tile_pool(name="fp", bufs=1))

        nc.gpsimd.dma_start(shard_b[:], emb_in[:])
        nc.gpsimd.collective_compute(
            "AllGather",
            mybir.AluOpType.bypass,
            replica_groups=[list(range(NCORES))],
            ins=[shard_b[:].opt()],
            outs=[gat[:].opt()],
        )
        # unpack 12-bit shards via SBUF. Only proven DVE patterns: contiguous
        # tensor_scalar outputs, and stride-2-out/contiguous-in copies.
        # Per core 1.6M elements; 5 sub-chunks of 320k elements each.
        AND = mybir.AluOpType.bitwise_and
        SHL = mybir.AluOpType.logical_shift_left
        gflat = gat[:].flatten()
        tflat = table[:].bitcast(u8).flatten()
        mp = ctx.enter_context(tc.tile_pool(name="mp", bufs=2))
        CBC = EMB_IN_ROWS * EMBED  # 2.4M input bytes per core
        HIB = VSHARD * EMBED       # 1.6M hi bytes per core
        for c in range(NCORES):
            for s in range(5):
                nib = mp.tile([128, 1250], u8, tag="nib")
                hi = mp.tile([128, 2500], u8, tag="hi")
                n_off = c * CBC + s * 128 * 1250
                h_off = c * CBC + HIB // 2 + s * 128 * 2500
                nc.sync.dma_start(
                    nib[:],
                    gflat[n_off : n_off + 128 * 1250].rearrange("(p w) -> p w", p=128),
                )
                nc.sync.dma_start(
                    hi[:],
                    gflat[h_off : h_off + 128 * 2500].rearrange("(p w) -> p w", p=128),
                )
                le = mp.tile([128, 1250], u8, tag="le")
                nc.vector.tensor_scalar(le[:], nib[:], 15, 4, AND, SHL)
                lodd = mp.tile([128, 1250], u8, tag="lodd")
                nc.vector.tensor_scalar(lodd[:], nib[:], 240, None, AND)
                lo2 = mp.tile([128, 2500], u8, tag="lo2")
                lo2v = lo2[:].rearrange("p (w two) -> p w two", two=2)
                nc.vector.tensor_copy(lo2v[:, :, 0:1], le[:].unsqueeze(2))
                nc.vector.tensor_copy(lo2v[:, :, 1:2], lodd[:].unsqueeze(2))
                mg = mp.tile([128, 5000], u8, tag="mg")
                mg3 = mg[:].rearrange("p (w two) -> p w two", two=2)
                nc.vector.tensor_copy(mg3[:, :, 0:1], lo2[:].unsqueeze(2))
                nc.vector.tensor_copy(mg3[:, :, 1:2], hi[:].unsqueeze(2))
                t_off = c * 2 * HIB + s * 128 * 5000
                nc.sync.dma_start(
                    tflat[t_off : t_off + 128 * 5000].rearrange("(p w) -> p w", p=128),
                    mg[:],
                )

        idsp_sb = const.tile([128, 3 * IDS_COLS], u8)
        nc.sync.dma_start(idsp_sb[:], ids[:])
        ids_sb = const.tile([128, IDS_COLS], i32)
        nc.vector.tensor_copy(ids_sb[:], idsp_sb[:, 0:IDS_COLS])
        idt = const.tile([128, IDS_COLS], i32, tag="idt")
        nc.vector.tensor_copy(idt[:], idsp_sb[:, IDS_COLS : 2 * IDS_COLS])
        nc.vector.tensor_scalar_mul(idt[:], idt[:], 256.0)
        nc.vector.tensor_add(ids_sb[:], ids_sb[:], idt[:])
        nc.vector.tensor_copy(idt[:], idsp_sb[:, 2 * IDS_COLS : 3 * IDS_COLS])
        nc.vector.tensor_scalar_mul(idt[:], idt[:], 65536.0)
        nc.vector.tensor_add(ids_sb[:], ids_sb[:], idt[:])
        qrep = const.tile([128, QROW_COLS], i32)
        nc.sync.dma_start(qrep[:], qrow[:].broadcast_to((128, QROW_COLS)))
        w_sb = const.tile([NK, 1], f32)
        nc.sync.dma_start(w_sb[:], wvec[:])
        ident = const.tile([128, 128], f16)
        make_identity(nc, ident[:])
        # per-k ones-selector matrices: sel_k[:, j] = 1.0 iff j == k
        sels = []
        for k in range(NK):
            sel = const.tile([128, NK], f32, tag=f"sel{k}")
            nc.vector.memset(sel[:], 0.0)
            nc.vector.memset(sel[:, k : k + 1], 1.0)
            sels.append(sel)
        # bias constants as [128,1] APs (float biases need pre-registered
        # const APs; only 0.0/1.0 exist)
        bias_tiles = {}
        for val in sorted({-50.0 * mu * mu for mu in _MUS}):
            bt = const.tile([128, 1], f32, tag=f"bias{val}")
            nc.vector.memset(bt[:], val)
            bias_tiles[val] = bt

        f_sb = fp.tile([1, 2 * BLOC], f32)

        for p in range(2):
            idbase = p * TILES_PER_PASS
            for sc in range(NSC):
                X = xp.tile([128, XCOLS], f32, tag="X")
                # ---- gather + transpose + sim matmuls for 8 groups ----
                for gl in range(GPS):
                    g = sc * GPS + gl
                    qe = gqe.tile([128, 128], f16, tag="qe")
                    qcol = idbase + DE_TILES + g
                    nc.gpsimd.indirect_dma_start(
                        out=qe[:],
                        out_offset=None,
                        in_=table[:],
                        in_offset=bass.IndirectOffsetOnAxis(
                            ap=ids_sb[:, qcol : qcol + 1], axis=0
                        ),
                    )
                    qeT_ps = tps.tile([128, 128], f16, tag="tps")
                    nc.tensor.transpose(qeT_ps[:], qe[:], ident[:])
                    qeT = qet.tile([128, 128], f16, tag="qeT")
                    nc.vector.tensor_copy(qeT[:], qeT_ps[:])

                    for h in range(2):
                        sim_ps = sps.tile([128, 128], f32, tag="sim")
                        for bs in range(4):
                            b = 4 * g + bs
                            dcol = idbase + 2 * b + h
                            de = gde.tile([128, 128], f16, tag="de")
                            nc.gpsimd.indirect_dma_start(
                                out=de[:],
                                out_offset=None,
                                in_=table[:],
                                in_offset=bass.IndirectOffsetOnAxis(
                                    ap=ids_sb[:, dcol : dcol + 1], axis=0
                                ),
                            )
                            deT_ps = tps.tile([128, 128], f16, tag="tps")
                            nc.tensor.transpose(deT_ps[:], de[:], ident[:])
                            deT = det.tile([128, 128], f16, tag="deT")
                            nc.vector.tensor_copy(deT[:], deT_ps[:])
                            nc.tensor.matmul(
                                sim_ps[:, 32 * bs : 32 * bs + 32],
                                lhsT=deT[:],
                                rhs=qeT[:, 32 * bs : 32 * bs + 32],
                                start=True,
                                stop=True,
                            )
                        nc.scalar.copy(
                            X[:, h * SCCOLS + gl * 128 : h * SCCOLS + gl * 128 + 128],
                            sim_ps[:],
                        )

                # ---- histogram over this super-chunk ----
                T1 = up.tile([128, XCOLS], f32, tag="T1")
                nc.vector.tensor_mul(T1[:], X[:], X[:])
                U = up.tile([128, XCOLS], f32, tag="U")
                nc.scalar.activation(U[:], T1[:], EXP, scale=-50.0)

                pooled_ps = pooled.tile([NK, 1024], f32, tag="pool")

                for k in range(NK):
                    P = pp.tile([128, XCOLS], f32, tag="P")
                    if k < 10:
                        mu = _MUS[k]
                        V = vp.tile([128, XCOLS], f32, tag="V")
                        nc.scalar.activation(
                            V[:], X[:], EXP, scale=100.0 * mu,
                            bias=bias_tiles[-50.0 * mu * mu][:],
                        )
                        nc.vector.tensor_mul(P[:], U[:], V[:])
                    else:
                        # exact bin: 1.0 where doc id == query id
                        for gl in range(GPS):
                            g = sc * GPS + gl
                            blk = ids_sb[
                                :, idbase + 8 * g : idbase + 8 * g + 8
                            ].rearrange("p (bs two) -> p two bs", two=2)
                            for h in range(2):
                                in0 = blk[:, h, :].unsqueeze(2).broadcast_to(
                                    (128, 4, 32)
                                )
                                in1 = qrep[
                                    :, p * 4096 + g * 128 : p * 4096 + g * 128 + 128
                                ].rearrange("q (bs c) -> q bs c", bs=4)
                                outp = P[
                                    :,
                                    h * SCCOLS + gl * 128 : h * SCCOLS + gl * 128 + 128,
                                ].rearrange("d (bs c) -> d bs c", bs=4)
                                nc.vector.tensor_tensor(outp, in0, in1, op=ISEQ)
                    for blk2 in range(2):
                        for h in range(2):
                            nc.tensor.matmul(
                                pooled_ps[:, blk2 * 512 : blk2 * 512 + 512],
                                lhsT=sels[k][:],
                                rhs=P[
                                    :,
                                    h * SCCOLS + blk2 * 512 : h * SCCOLS + blk2 * 512 + 512,
                                ],
                                start=(k == 0 and h == 0),
                                stop=(k == NK - 1 and h == 1),
                            )

                # ---- log1p, mlp dot, query-sum ----
                L = lp.tile([NK, 1024], f32, tag="L")
                nc.scalar.activation(L[:, 0:512], pooled_ps[:, 0:512], LOG, bias=1.0)
                nc.scalar.activation(L[:, 512:1024], pooled_ps[:, 512:1024], LOG, bias=1.0)
                for blk2 in range(2):
                    logit_ps = lgt.tile([1, 512], f32, tag="logit")
                    nc.tensor.matmul(
                        logit_ps[:],
                        lhsT=w_sb[:],
                        rhs=L[:, blk2 * 512 : blk2 * 512 + 512],
                        start=True,
                        stop=True,
                    )
                    base = p * BLOC + sc * 32 + blk2 * 16
                    nc.vector.tensor_reduce(
                        f_sb[:, base : base + 16],
                        logit_ps[:].rearrange("o (b q) -> o b q", q=QLEN),
                        axis=AXX,
                        op=ADD,
                    )

        # ---- sigmoid(f1 - f2) ----
        diff = fp.tile([1, BLOC], f32)
        nc.vector.tensor_sub(diff[:], f_sb[:, 0:BLOC], f_sb[:, BLOC : 2 * BLOC])
        en = fp.tile([1, BLOC], f32)
        nc.scalar.activation(en[:], diff[:], EXP, scale=-1.0)
        enp1 = fp.tile([1, BLOC], f32)
        nc.vector.tensor_scalar_add(enp1[:], en[:], 1.0)
        sig = fp.tile([1, BLOC], f32)
        nc.vector.reciprocal(sig[:], enp1[:])
        nc.sync.dma_start(out[:], sig[:])

    nc.finalize()
    return nc, emb_in.name, ids.name, qrow.name, wvec.name, out.name


_CACHE = {}


def _get_nc():
    if "nc" not in _CACHE:
        _CACHE["nc"] = _build_nc()
    return _CACHE["nc"]


def _install_fast_pjrt():
    """Memoize the jit executable inside bass2jax.run_bass_via_pjrt.

    The stock version rebuilds jax.jit(shard_map(closure)) per call, paying a
    full retrace (~0.4s) every time. Build it once per Bass module and reuse.
    """
    import jax
    import concourse.bass2jax as b2j
    import concourse.mybir as mybir
    from concourse.bass2jax import (
        _bass_exec_p,
        install_neuronx_cc_hook,
        partition_id_tensor,
    )
    from jax.experimental.shard_map import shard_map
    from jax.sharding import Mesh, PartitionSpec

    if getattr(b2j.run_bass_via_pjrt, "_knrm_fast", False):
        return
    orig = b2j.run_bass_via_pjrt
    cache = {}

    def fast(nc, in_maps, n_cores):
        if n_cores == 1 or nc.dbg_addr is not None:
            return orig(nc, in_maps, n_cores)
        key = (id(nc), n_cores)
        if key not in cache:
            install_neuronx_cc_hook()
            partition_name = (
                nc.partition_id_tensor.name if nc.partition_id_tensor else None
            )
            in_names, out_names, out_avals, zero_shapes = [], [], [], []
            for alloc in nc.m.functions[0].allocations:
                if not isinstance(alloc, mybir.MemoryLocationSet):
                    continue
                name = alloc.memorylocations[0].name
                if alloc.kind == "ExternalInput":
                    if name != partition_name:
                        in_names.append(name)
                elif alloc.kind == "ExternalOutput":
                    out_names.append(name)
                    shape = tuple(alloc.tensor_shape)
                    dtype = mybir.dt.np(alloc.dtype)
                    out_avals.append(jax.core.ShapedArray(shape, dtype))
                    zero_shapes.append((shape, dtype))
            n_params, n_outs = len(in_names), len(out_names)
            all_in = list(in_names) + out_names + (
                [partition_name] if partition_name else []
            )

            def _body(*args):
                operands = list(args)
                if partition_name is not None:
                    operands.append(partition_id_tensor())
                outs = _bass_exec_p.bind(
                    *operands,
                    out_avals=tuple(out_avals),
                    in_names=tuple(all_in),
                    out_names=tuple(out_names),
                    lowering_input_output_aliases=(),
                    sim_require_finite=True,
                    sim_require_nnan=True,
                    nc=nc,
                )
                return tuple(outs)

            mesh = Mesh(np.asarray(jax.devices()[:n_cores]), ("core",))
            sharded = jax.jit(
                shard_map(
                    _body,
                    mesh=mesh,
                    in_specs=(PartitionSpec("core"),) * (n_params + n_outs),
                    out_specs=(PartitionSpec("core"),) * n_outs,
                    check_rep=False,
                ),
                donate_argnums=tuple(range(n_params, n_params + n_outs)),
                keep_unused=True,
            )
            cache[key] = (sharded, in_names, out_names, out_avals, zero_shapes)
        sharded, in_names, out_names, out_avals, zero_shapes = cache[key]

        def _concat(pieces):
            # if the pieces are already adjacent views of one buffer (built
            # that way by kernel()), skip the copy
            try:
                first, last = pieces[0], pieces[-1]
                if all(p.flags.c_contiguous for p in pieces):
                    starts = [p.__array_interface__["data"][0] for p in pieces]
                    ok = all(
                        starts[i] + pieces[i].nbytes == starts[i + 1]
                        for i in range(len(pieces) - 1)
                    )
                    if ok:
                        base = first
                        while base.base is not None and isinstance(
                            base.base, np.ndarray
                        ):
                            base = base.base
                        total0 = sum(p.shape[0] for p in pieces)
                        off = starts[0] - base.__array_interface__["data"][0]
                        flat = base.reshape(-1).view(first.dtype)
                        n = total0 * int(np.prod(first.shape[1:], dtype=np.int64))
                        start_el = off // first.dtype.itemsize
                        return flat[start_el : start_el + n].reshape(
                            (total0,) + first.shape[1:]
                        )
            except Exception:
                pass
            return np.concatenate(pieces, axis=0)

        concat_in = [
            _concat([np.asarray(m[nm]) for m in in_maps]) for nm in in_names
        ]
        concat_zeros = [
            np.zeros((n_cores * s[0], *s[1:]), d) for (s, d) in zero_shapes
        ]
        # AOT-compile once and dispatch through the compiled executable to
        # skip per-call jit argument-processing overhead
        ck = ("aot", key)
        if ck not in cache:
            try:
                cache[ck] = sharded.lower(*concat_in, *concat_zeros).compile()
            except Exception:
                cache[ck] = None
        compiled = cache[ck]
        if compiled is not None:
            outs = compiled(*concat_in, *concat_zeros)
        else:
            outs = sharded(*concat_in, *concat_zeros)
        return [
            {
                nm: np.asarray(outs[i]).reshape(n_cores, *out_avals[i].shape)[c]
                for i, nm in enumerate(out_names)
            }
            for c in range(n_cores)
        ]

    fast._knrm_fast = True
    b2j.run_bass_via_pjrt = fast


def _warmup():
    """Compile the NEFF + init jax/axon/comm at import time so kernel() only
    pays host prep + input transfer + execution."""
    from concourse.bass_utils import run_bass_kernel_spmd

    _install_fast_pjrt()
    nc, ename, iname, qname, wname, oname = _get_nc()
    emb0 = np.zeros((3 * VSHARD // 2, EMBED), dtype=np.uint8)
    ids0 = np.zeros((128, 3 * IDS_COLS), dtype=np.uint8)
    qrow0 = np.zeros((1, QROW_COLS), dtype=np.int32)
    w0 = np.zeros((NK, 1), dtype=np.float32)
    in_maps = [
        {ename: emb0, iname: ids0, qname: qrow0, wname: w0}
        for c in range(NCORES)
    ]
    run_bass_kernel_spmd(nc, in_maps, core_ids=list(range(NCORES)))


try:
    _warmup()
except Exception:
    pass


def _prep_planes(emb, big, c):
    """Normalize rows of core c's vocab shard, round fp16 to 6 mantissa bits,
    split into lo/hi byte planes in big[c]."""
    sl = slice(c * VSHARD, (c + 1) * VSHARD)
    blk = emb[sl]
    ss = np.einsum("ij,ij->i", blk, blk)
    np.sqrt(ss, out=ss)
    np.reciprocal(ss, out=ss)
    n16 = np.empty((VSHARD, EMBED), dtype=np.float16)
    np.multiply(blk, ss[:, None], out=n16, casting="unsafe")
    u = n16.view(np.uint16)
    # round-to-nearest-even to 6 mantissa bits (values < 1.0, no overflow);
    # bits [3:0] are dropped by the extraction below, no mask needed
    t = u >> 4
    t &= 1
    t += 7
    u += t
    uf = u.reshape(-1)
    half = VSHARD * EMBED // 2
    bc = big[c].reshape(-1)
    # packed nibble byte = (even>>4 & 15) | (odd & 240)
    e = uf[0::2] >> 4
    e &= 15
    o = uf[1::2] & 240
    e |= o
    bc[:half] = e.astype(np.uint8)
    uf >>= 8
    bc[half:] = uf.astype(np.uint8)


def kernel(emb, mlp_w, mlp_b, query1, doc1, query2, doc2):
    from concourse.bass_utils import run_bass_kernel_spmd

    emb = np.asarray(emb, dtype=np.float32)
    big = np.empty((NCORES, 3 * VSHARD // 2, EMBED), dtype=np.uint8)
    for c in range(NCORES):
        _prep_planes(emb, big, c)

    w = np.asarray(mlp_w, dtype=np.float32).reshape(NK, 1)
    q1 = np.asarray(query1).astype(np.int32)
    d1 = np.asarray(doc1).astype(np.int32)
    q2 = np.asarray(query2).astype(np.int32)
    d2 = np.asarray(doc2).astype(np.int32)

    nc, ename, iname, qname, wname, oname = _get_nc()

    # build per-core inputs as adjacent slices of parent arrays so the fast
    # pjrt path can skip the 22MB concat copy; ids laid out for all cores at
    # once (qrow is just the flattened query array)
    idsv = np.empty((NCORES, 128, IDS_COLS), dtype=np.int32)
    for p, (q, d) in enumerate(((q1, d1), (q2, d2))):
        base = p * TILES_PER_PASS
        idsv[:, :, base : base + DE_TILES] = (
            d.reshape(NCORES, BLOC, 2, 128).transpose(0, 3, 1, 2)
            .reshape(NCORES, 128, DE_TILES)
        )
        idsv[:, :, base + DE_TILES : base + TILES_PER_PASS] = (
            q.reshape(NCORES, NGRP, 4, QLEN).transpose(0, 2, 3, 1)
            .reshape(NCORES, 128, QE_TILES)
        )
    idsb = idsv.view(np.uint8).reshape(NCORES, 128, IDS_COLS, 4)
    all_ids = np.empty((NCORES * 128, 3 * IDS_COLS), dtype=np.uint8)
    av = all_ids.reshape(NCORES, 128, 3, IDS_COLS)
    av[:, :, 0] = idsb[:, :, :, 0]
    av[:, :, 1] = idsb[:, :, :, 1]
    av[:, :, 2] = idsb[:, :, :, 2]
    all_q = np.empty((NCORES, QROW_COLS), dtype=np.int32)
    all_q[:, : NGRP * 128] = q1.reshape(NCORES, NGRP * 128)
    all_q[:, NGRP * 128 :] = q2.reshape(NCORES, NGRP * 128)
    in_maps = [
        {
            ename: big[c],
            iname: all_ids[c * 128 : (c + 1) * 128],
            qname: all_q[c : c + 1],
            wname: w,
        }
        for c in range(NCORES)
    ]

    trace = os.environ.get("KNRM_TRACE") == "1"
    try:
        res = run_bass_kernel_spmd(
            nc, in_maps, core_ids=list(range(NCORES)), trace=trace,
            trace_cores=[0] if trace else None,
        )
    except Exception:
        # transient axon/device hiccup: retry once
        res = run_bass_kernel_spmd(
            nc, in_maps, core_ids=list(range(NCORES)), trace=trace,
            trace_cores=[0] if trace else None,
        )
    global LAST_RESULT
    LAST_RESULT = res
    out = np.concatenate([res.results[c][oname].reshape(BLOC) for c in range(NCORES)])
    # mlp_b cancels in logits_1 - logits_2; output float32 [B, 1]
    return out.reshape(B, 1).astype(np.float32)
